# revision 2
# baseline (speedup 1.0000x reference)
"""Trainium2 Bass kernel for Autoformer-style autocorrelation attention.

Math (matches the reference nn.Module):
    top_k = int(log(L)) = 6
    mean_value[b, l] = corr[b].mean(over H, C)                     # [B, L]
    idx = top_k(mean_value.mean(over B))                           # [6]
    w = softmax(mean_value[:, idx], axis=-1)                       # [B, 6]
    out[b, h, c, l] = sum_k w[b, k] * values[b, h, c, (l+idx_k)%L]

Strategy: data-parallel over B (4 batches per core on 8 cores).

Launch 1 reduces corr over (H, C) per batch on-device via ones-matmuls
over the partition axis.  corr is sent as fp16: the quantization error on
the means (~1e-5) is far below the 4.8e-4 top-k selection margin measured
on this distribution, and it halves launch-1 HBM traffic.  The [32, L]
sums return to host, where the tiny top-k + softmax glue runs.

Launch 2 bakes the 6 indices in as static SBUF column windows and emits
the output in fp16 (host casts to fp32; adds <=4.9e-4 relative error
against the 2e-2 gate, and halves the write traffic).  The six shift
terms are spread across three engines so no engine exceeds the DMA pace:

  - ACT seeds each PSUM tile with term 4 (activation Copy, scale=w4 AP,
    writing PSUM directly),
  - PE accumulates terms 0-3 on top with start=False diag-matmuls (a
    warmup burst first touches every PSUM bank with start=True so the
    has_written bits are set and the seeded values survive),
  - DVE finishes with one fused scalar_tensor_tensor pass per tile:
    out16 = (shift5(v) * w5) + psum, casting to fp16 on the way out.

Diag matrices (w[b,k] * I) are built on-device from a 32KB identity
upload, so launch-2 input DMA is just values fp16 + a few KB.  Per-batch
weights enter through an input tensor so one compiled NEFF is SPMD
across all 8 cores.
"""

import math

import numpy as np

_B, _H, _C, _L = 32, 8, 64, 1024
_NCORES = 8
_BLOC = _B // _NCORES  # batches per core
_R = _H * _C           # rows per batch
_PART = 128
_TPB = _R // _PART     # SBUF tiles per batch
_TOPK = int(math.log(_L))  # 6
_NPE = 4               # shift terms handled by the tensor engine (k=0..3)
_HALF = 512            # PSUM bank width in fp32


def _build_phase1():
    import concourse.bacc as bacc
    import concourse.mybir as mybir
    import concourse.tile as tile

    f32 = mybir.dt.float32
    f16 = mybir.dt.float16
    nc = bacc.Bacc("TRN2", target_bir_lowering=False, debug=False,
                   enable_partition_id=False)
    corr_d = nc.dram_tensor("corr_sh", [_BLOC, _R, _L], f16, kind="ExternalInput").ap()
    sums_d = nc.dram_tensor("sums", [1, _BLOC * _L], f32, kind="ExternalOutput").ap()

    with tile.TileContext(nc) as tc:
        with (
            tc.tile_pool(name="io", bufs=6) as io_pool,
            tc.tile_pool(name="const", bufs=1) as const_pool,
            tc.tile_pool(name="acc", bufs=1) as acc_pool,
            tc.tile_pool(name="ps", bufs=3, space="PSUM") as ps_pool,
        ):
            ones = const_pool.tile([_PART, _HALF], f16)
            nc.vector.memset(ones[:], 1.0)
            outs = acc_pool.tile([1, _BLOC * _L], f32)
            # HAM warmup: junk matmuls so the PE clock ramps while the entry
            # barrier + first DMA latency play out
            wps = ps_pool.tile([_PART, _HALF], f32, tag="wps", name="wps", bufs=1)
            for _ in range(10):
                nc.tensor.matmul(wps[:], ones[:, 0:_PART], ones[:],
                                 start=True, stop=True)
            for b in range(_BLOC):
                pss = [ps_pool.tile([_PART, _HALF], f32, tag=f"ps{h}", name=f"ps{h}")
                       for h in range(2)]
                for t2 in range(_TPB // 2):
                    # two row-blocks per DMA: [128, 2L] tile, halving DMA count
                    vt = io_pool.tile([_PART, 2 * _L], f16, tag="vt")
                    src_ap = corr_d[b, t2 * 2 * _PART:(t2 + 1) * 2 * _PART, :]
                    nc.sync.dma_start(
                        vt[:].rearrange("p (u l) -> p u l", u=2),
                        src_ap.rearrange("(u p) l -> p u l", p=_PART))
                    for u in range(2):
                        for h in range(2):
                            nc.tensor.matmul(
                                pss[h][:],
                                ones[:, 0:_PART],
                                vt[:, u * _L + h * _HALF:u * _L + (h + 1) * _HALF],
                                start=(t2 == 0 and u == 0),
                                stop=(t2 == _TPB // 2 - 1 and u == 1),
                            )
                for h in range(2):
                    o0 = b * _L + h * _HALF
                    nc.scalar.copy(outs[0:1, o0:o0 + _HALF], pss[h][0:1, :])
                nc.scalar.dma_start(
                    sums_d[0:1, b * _L:(b + 1) * _L],
                    outs[0:1, b * _L:(b + 1) * _L])
    nc.compile()
    return nc


def _wrap_pieces(s):
    """Split the circular window [s, s+L) into contiguous source pieces.

    Returns [(dst_off, n, src_off), ...] with sum(n) == L.
    """
    if s == 0:
        return [(0, _L, 0)]
    return [(0, _L - s, s), (_L - s, s, 0)]


def _build_phase2(idx):
    import concourse.bacc as bacc
    import concourse.mybir as mybir
    import concourse.tile as tile

    f32 = mybir.dt.float32
    f16 = mybir.dt.float16
    alu = mybir.AluOpType
    act_copy = mybir.ActivationFunctionType.Copy

    nc = bacc.Bacc("TRN2", target_bir_lowering=False, debug=False,
                   enable_partition_id=False)
    vals_d = nc.dram_tensor("vals", [_BLOC, _R, _L], f16, kind="ExternalInput").ap()
    wsb_d = nc.dram_tensor("wsb", [_PART, _BLOC * _TOPK], f32, kind="ExternalInput").ap()
    eye_d = nc.dram_tensor("eye", [_PART, _PART], f16, kind="ExternalInput").ap()
    out_d = nc.dram_tensor("out_sh", [_BLOC, _R, _L], f16, kind="ExternalOutput").ap()

    with tile.TileContext(nc) as tc:
        with (
            tc.tile_pool(name="const", bufs=1) as const_pool,
            tc.tile_pool(name="v16", bufs=8) as v16_pool,
            tc.tile_pool(name="out", bufs=4) as out_pool,
            tc.tile_pool(name="ps", bufs=4, space="PSUM") as ps_pool,
        ):
            # HAM warmup on an independent memset tile; the warmup matmuls
            # also visit every PSUM pool slot with start=True so each bank's
            # has_written bits are set before the seeded accumulation below.
            wones = const_pool.tile([_PART, _HALF], f16)
            nc.vector.memset(wones[:], 1.0)
            for _ in range(5):
                wp = ps_pool.tile([_PART, _L], f32, tag="ps", name="ps")
                for h in range(2):
                    nc.tensor.matmul(wp[:, h * _HALF:(h + 1) * _HALF],
                                     wones[:, 0:_PART], wones[:],
                                     start=True, stop=True)
            w_t = const_pool.tile([_PART, _BLOC * _TOPK], f32)
            nc.sync.dma_start(w_t[:], wsb_d[:])
            eye = const_pool.tile([_PART, _PART], f16)
            nc.sync.dma_start(eye[:], eye_d[:])
            # weighted diag stationaries built on DVE: diag[b,k] = w[b,k]*I
            diag = const_pool.tile([_PART, _BLOC * _NPE * _PART], f16)
            for b in range(_BLOC):
                for k in range(_NPE):
                    dof = (b * _NPE + k) * _PART
                    nc.vector.tensor_scalar_mul(
                        diag[:, dof:dof + _PART], eye[:],
                        w_t[:, b * _TOPK + k:b * _TOPK + k + 1])

            for b in range(_BLOC):
                w4 = w_t[:, b * _TOPK + _NPE:b * _TOPK + _NPE + 1]
                w5 = w_t[:, b * _TOPK + _NPE + 1:b * _TOPK + _NPE + 2]
                for t in range(_TPB):
                    vt16 = v16_pool.tile([_PART, _L], f16, tag="vt16")
                    nc.sync.dma_start(
                        vt16[:], vals_d[b, t * _PART:(t + 1) * _PART, :])

                    ps = ps_pool.tile([_PART, _L], f32, tag="ps", name="ps")
                    # ACT seeds the PSUM tile with term 4 (engine write spans
                    # both banks; only matmuls care about bank boundaries)
                    for (d0, n, s0) in _wrap_pieces(idx[_NPE]):
                        nc.scalar.activation(ps[:, d0:d0 + n], vt16[:, s0:s0 + n],
                                             act_copy, scale=w4)

                    # PE accumulates terms 0..3 on top: start=False keeps the
                    # seed (banks' has_written bits were set by the warmup /
                    # previous use of the slot)
                    pieces = []
                    for k in range(_NPE):
                        dof = (b * _NPE + k) * _PART
                        for h in range(2):
                            s = (idx[k] + h * _HALF) % _L
                            n1 = min(_HALF, _L - s)
                            pieces.append((dof, h * _HALF, n1, s))
                            if n1 < _HALF:
                                pieces.append((dof, h * _HALF + n1, _HALF - n1, 0))
                    for pi, (dof, o0, n, s) in enumerate(pieces):
                        nc.tensor.matmul(
                            ps[:, o0:o0 + n], diag[:, dof:dof + _PART],
                            vt16[:, s:s + n],
                            start=False, stop=(pi == len(pieces) - 1),
                            skip_group_check=True,
                        )

                    # DVE: fused term 5 + merge + fp16 cast:
                    #   ot = (shift5(v) * w5) + psum
                    ot = out_pool.tile([_PART, _L], f16, tag="ot")
                    for (d0, n, s0) in _wrap_pieces(idx[_NPE + 1]):
                        nc.vector.scalar_tensor_tensor(
                            ot[:, d0:d0 + n],
                            vt16[:, s0:s0 + n],
                            w5,
                            ps[:, d0:d0 + n],
                            op0=alu.mult,
                            op1=alu.add,
                        )
                    nc.sync.dma_start(out_d[b, t * _PART:(t + 1) * _PART, :], ot[:])
    nc.compile()
    return nc


def _run_spmd(nc, in_maps, **kwargs):
    from concourse import bass_utils

    return bass_utils.run_bass_kernel_spmd(
        nc, in_maps, core_ids=list(range(_NCORES)), **kwargs
    )


def kernel(values: np.ndarray, corr: np.ndarray, _collect=None) -> np.ndarray:
    assert values.shape == (_B, _H, _C, _L) and corr.shape == (_B, _H, _C, _L)
    corr16 = np.ascontiguousarray(
        np.asarray(corr, dtype=np.float32).reshape(_B, _R, _L), dtype=np.float16
    )
    vals16 = np.ascontiguousarray(
        np.asarray(values, dtype=np.float32).reshape(_B, _R, _L), dtype=np.float16
    )

    # ---- launch 1: per-batch sums of corr over (H, C) ----
    nc1 = _build_phase1()
    in1 = [
        {"corr_sh": corr16[c * _BLOC:(c + 1) * _BLOC]}
        for c in range(_NCORES)
    ]
    res1 = _run_spmd(nc1, in1, **(_collect.kwargs(1) if _collect else {}))
    if _collect is not None:
        _collect.add(1, nc1, res1)
    sums = np.concatenate(
        [r["sums"].reshape(_BLOC, _L) for r in res1.results], axis=0
    )  # [B, L]

    # ---- host glue: top-k indices + softmax weights (tiny) ----
    mean_value = sums / np.float32(_R)                       # [B, L]
    g = mean_value.astype(np.float64).mean(axis=0)           # [L]
    idx = np.argsort(-g, kind="stable")[:_TOPK].astype(np.int64)
    wsel = mean_value[:, idx].astype(np.float32)             # [B, 6]
    e = np.exp(wsel - wsel.max(axis=-1, keepdims=True))
    w = (e / e.sum(axis=-1, keepdims=True)).astype(np.float32)

    # ---- launch 2: weighted shifted-gather combine ----
    nc2 = _build_phase2([int(i) for i in idx])
    eye = np.eye(_PART, dtype=np.float16)
    in2 = []
    for c in range(_NCORES):
        wloc = w[c * _BLOC:(c + 1) * _BLOC]                  # [BLOC, 6]
        wsb = np.ascontiguousarray(
            np.broadcast_to(wloc.reshape(-1)[None, :], (_PART, _BLOC * _TOPK)),
            dtype=np.float32,
        )
        in2.append({
            "vals": vals16[c * _BLOC:(c + 1) * _BLOC],
            "wsb": wsb,
            "eye": eye,
        })
    res2 = _run_spmd(nc2, in2, **(_collect.kwargs(2) if _collect else {}))
    if _collect is not None:
        _collect.add(2, nc2, res2)
    out = np.concatenate([np.asarray(r["out_sh"]) for r in res2.results], axis=0)
    return out.reshape(_B, _H, _C, _L).astype(np.float32)


# revision 7
# speedup vs baseline: 1.0129x; 1.0129x over previous
"""Trainium2 Bass kernel for Autoformer-style autocorrelation attention.

Math (matches the reference nn.Module):
    top_k = int(log(L)) = 6
    mean_value[b, l] = corr[b].mean(over H, C)                     # [B, L]
    idx = top_k(mean_value.mean(over B))                           # [6]
    w = softmax(mean_value[:, idx], axis=-1)                       # [B, 6]
    out[b, h, c, l] = sum_k w[b, k] * values[b, h, c, (l+idx_k)%L]

Strategy: data-parallel over B (4 batches per core on 8 cores).

Launch 1 reduces corr over (H, C) per batch on-device via ones-matmuls
over the partition axis.  corr is sent as fp16: the quantization error on
the means (~1e-5) is far below the 4.8e-4 top-k selection margin measured
on this distribution, and it halves launch-1 HBM traffic.  The [32, L]
sums return to host, where the tiny top-k + softmax glue runs.

Launch 2 bakes the 6 indices in as static SBUF column windows and emits
the output in fp16 (host casts to fp32; adds <=4.9e-4 relative error
against the 2e-2 gate, and halves the write traffic).  The six shift
terms are split so no engine exceeds the DMA pace: four run on PE as
diag-weighted matmuls accumulating in PSUM, and the last two are fused
into the two DVE scalar_tensor_tensor passes that drain PSUM:

    u16 = (shiftA(v) * wA) + psum      # fp16 out, 2x DVE fast path
    ot  = (shiftB(v) * wB) + u16       # all-SBUF fp16

DVE pieces are split at PSUM bank boundaries (in-bank PSUM reads run
~5x faster than bank-crossing ones) and DVE gets the even shifts.

Diag matrices (w[b,k] * I) are built on-device from a 32KB identity
upload, so launch-2 input DMA is just values fp16 + a few KB.  Per-batch
weights enter through an input tensor so one compiled NEFF is SPMD
across all 8 cores.
"""

import math

import numpy as np

_B, _H, _C, _L = 32, 8, 64, 1024
_NCORES = 8
_BLOC = _B // _NCORES  # batches per core
_R = _H * _C           # rows per batch
_PART = 128
_TPB = _R // _PART     # SBUF tiles per batch
_TOPK = int(math.log(_L))  # 6
_NPE = 4               # shift terms handled by the tensor engine (k=0..3)
_HALF = 512            # PSUM bank width in fp32


def _build_phase1():
    import concourse.bacc as bacc
    import concourse.mybir as mybir
    import concourse.tile as tile

    f32 = mybir.dt.float32
    f16 = mybir.dt.float16
    nc = bacc.Bacc("TRN2", target_bir_lowering=False, debug=False,
                   enable_partition_id=False)
    corr_d = nc.dram_tensor("corr_sh", [_BLOC, _R, _L], f16, kind="ExternalInput").ap()
    sums_d = nc.dram_tensor("sums", [1, _BLOC * _L], f32, kind="ExternalOutput").ap()

    with tile.TileContext(nc) as tc:
        with (
            tc.tile_pool(name="io", bufs=6) as io_pool,
            tc.tile_pool(name="const", bufs=1) as const_pool,
            tc.tile_pool(name="acc", bufs=1) as acc_pool,
            tc.tile_pool(name="ps", bufs=3, space="PSUM") as ps_pool,
        ):
            ones = const_pool.tile([_PART, _HALF], f16)
            nc.vector.memset(ones[:], 1.0)
            outs = acc_pool.tile([1, _BLOC * _L], f32)
            # HAM warmup: a dense junk-matmul burst so the PE clock ramps to
            # full while the entry barrier + first DMA latency play out.  The
            # short [128,128] matmuls keep the activity duty cycle high.
            wps = ps_pool.tile([_PART, _HALF], f32, tag="wps", name="wps", bufs=1)
            for _ in range(6):
                nc.tensor.matmul(wps[:], ones[:, 0:_PART], ones[:],
                                 start=True, stop=True)
            for _ in range(18):
                nc.tensor.matmul(wps[:, 0:_PART], ones[:, 0:_PART],
                                 ones[:, 0:_PART], start=True, stop=True)
            for b in range(_BLOC):
                pss = [ps_pool.tile([_PART, _HALF], f32, tag=f"ps{h}", name=f"ps{h}")
                       for h in range(2)]
                for t2 in range(_TPB // 2):
                    # two row-blocks per DMA: [128, 2L] tile, halving DMA count
                    vt = io_pool.tile([_PART, 2 * _L], f16, tag="vt")
                    src_ap = corr_d[b, t2 * 2 * _PART:(t2 + 1) * 2 * _PART, :]
                    nc.sync.dma_start(
                        vt[:].rearrange("p (u l) -> p u l", u=2),
                        src_ap.rearrange("(u p) l -> p u l", p=_PART))
                    for u in range(2):
                        for h in range(2):
                            nc.tensor.matmul(
                                pss[h][:],
                                ones[:, 0:_PART],
                                vt[:, u * _L + h * _HALF:u * _L + (h + 1) * _HALF],
                                start=(t2 == 0 and u == 0),
                                stop=(t2 == _TPB // 2 - 1 and u == 1),
                            )
                for h in range(2):
                    o0 = b * _L + h * _HALF
                    nc.scalar.copy(outs[0:1, o0:o0 + _HALF], pss[h][0:1, :])
                nc.scalar.dma_start(
                    sums_d[0:1, b * _L:(b + 1) * _L],
                    outs[0:1, b * _L:(b + 1) * _L])
    nc.compile()
    return nc


def _wrap_pieces(s):
    """Split the circular window [s, s+L) into contiguous source pieces.

    Returns [(dst_off, n, src_off), ...] with sum(n) == L.
    """
    if s == 0:
        return [(0, _L, 0)]
    return [(0, _L - s, s), (_L - s, s, 0)]


def _build_phase2(idx):
    import concourse.bacc as bacc
    import concourse.mybir as mybir
    import concourse.tile as tile

    f32 = mybir.dt.float32
    f16 = mybir.dt.float16
    alu = mybir.AluOpType

    # Four terms run on PE as diag-matmuls; the other two are fused into the
    # two DVE scalar_tensor_tensor passes.  DVE's fp16 2x fast path prefers
    # even source offsets, so give DVE even shifts when available.
    evens = [k for k in range(_TOPK) if idx[k] % 2 == 0]
    odds = [k for k in range(_TOPK) if idx[k] % 2 == 1]
    kdve = (evens + odds)[:2]
    kpe = [k for k in range(_TOPK) if k not in kdve]
    assert len(kpe) == _NPE

    nc = bacc.Bacc("TRN2", target_bir_lowering=False, debug=False,
                   enable_partition_id=False)
    vals_d = nc.dram_tensor("vals", [_BLOC, _R, _L], f16, kind="ExternalInput").ap()
    wsb_d = nc.dram_tensor("wsb", [_PART, _BLOC * _TOPK], f32, kind="ExternalInput").ap()
    eye_d = nc.dram_tensor("eye", [_PART, _PART], f16, kind="ExternalInput").ap()
    out_d = nc.dram_tensor("out_sh", [_BLOC, _R, _L], f16, kind="ExternalOutput").ap()

    with tile.TileContext(nc) as tc:
        with (
            tc.tile_pool(name="const", bufs=1) as const_pool,
            tc.tile_pool(name="v16", bufs=8) as v16_pool,
            tc.tile_pool(name="mid", bufs=4) as mid_pool,
            tc.tile_pool(name="out", bufs=4) as out_pool,
            tc.tile_pool(name="ps", bufs=4, space="PSUM") as ps_pool,
        ):
            w_t = const_pool.tile([_PART, _BLOC * _TOPK], f32)
            nc.sync.dma_start(w_t[:], wsb_d[:])
            eye = const_pool.tile([_PART, _PART], f16)
            nc.sync.dma_start(eye[:], eye_d[:])
            # HAM warmup: a dense burst of junk matmuls (a few full-bank ones
            # plus many short ones) keeps the PE duty cycle high so the clock
            # ramps to full before the real stream begins.
            wones = const_pool.tile([_PART, _HALF], f16)
            nc.vector.memset(wones[:], 1.0)
            wp = ps_pool.tile([_PART, _L], f32, tag="ps", name="ps")
            for h in range(2):
                nc.tensor.matmul(wp[:, h * _HALF:(h + 1) * _HALF],
                                 wones[:, 0:_PART], wones[:],
                                 start=True, stop=True)
            for _ in range(20):
                nc.tensor.matmul(wp[:, 0:_PART], wones[:, 0:_PART],
                                 wones[:, 0:_PART], start=True, stop=True)
            # weighted diag stationaries built on DVE: diag[b,j] = w[b,kpe[j]]*I
            diag = const_pool.tile([_PART, _BLOC * _NPE * _PART], f16)
            for b in range(_BLOC):
                for j, k in enumerate(kpe):
                    dof = (b * _NPE + j) * _PART
                    nc.vector.tensor_scalar_mul(
                        diag[:, dof:dof + _PART], eye[:],
                        w_t[:, b * _TOPK + k:b * _TOPK + k + 1])

            for b in range(_BLOC):
                wA = w_t[:, b * _TOPK + kdve[0]:b * _TOPK + kdve[0] + 1]
                wB = w_t[:, b * _TOPK + kdve[1]:b * _TOPK + kdve[1] + 1]
                for t in range(_TPB):
                    vt16 = v16_pool.tile([_PART, _L], f16, tag="vt16")
                    nc.sync.dma_start(
                        vt16[:], vals_d[b, t * _PART:(t + 1) * _PART, :])

                    # PE: 4 diag-weighted shift terms accumulate per PSUM
                    # bank; bank A pieces first so DVE can start on bank A
                    # while PE still works on bank B.
                    ps = ps_pool.tile([_PART, _L], f32, tag="ps", name="ps")
                    pieces = {0: [], 1: []}
                    for j, k in enumerate(kpe):
                        dof = (b * _NPE + j) * _PART
                        for h in range(2):
                            s = (idx[k] + h * _HALF) % _L
                            n1 = min(_HALF, _L - s)
                            pieces[h].append((dof, h * _HALF, n1, s))
                            if n1 < _HALF:
                                pieces[h].append((dof, h * _HALF + n1, _HALF - n1, 0))
                    for h in range(2):
                        for pi, (dof, o0, n, s) in enumerate(pieces[h]):
                            nc.tensor.matmul(
                                ps[:, o0:o0 + n], diag[:, dof:dof + _PART],
                                vt16[:, s:s + n],
                                start=(pi == 0), stop=(pi == len(pieces[h]) - 1),
                            )

                    # DVE pass 1: u16 = (shiftA(v) * wA) + psum, fp16 out.
                    # Pieces split at the source wrap AND at the PSUM bank
                    # boundary: an in-bank PSUM read runs ~5x faster than one
                    # crossing banks (measured 0.55 vs 2.84 ns/col).
                    u16 = mid_pool.tile([_PART, _L], f16, tag="u16")
                    sA = idx[kdve[0]]
                    cuts = sorted({0, _HALF, _L} | ({_L - sA} if sA else set()))
                    for d0, d1 in zip(cuts, cuts[1:]):
                        s0 = (d0 + sA) % _L
                        nc.vector.scalar_tensor_tensor(
                            u16[:, d0:d1], vt16[:, s0:s0 + (d1 - d0)], wA,
                            ps[:, d0:d1], op0=alu.mult, op1=alu.add)

                    # DVE pass 2: ot = (shiftB(v) * wB) + u16, all-SBUF fp16.
                    ot = out_pool.tile([_PART, _L], f16, tag="ot")
                    sB = idx[kdve[1]]
                    for (d0, n, s0) in _wrap_pieces(sB):
                        nc.vector.scalar_tensor_tensor(
                            ot[:, d0:d0 + n], vt16[:, s0:s0 + n], wB,
                            u16[:, d0:d0 + n], op0=alu.mult, op1=alu.add)
                    nc.scalar.dma_start(out_d[b, t * _PART:(t + 1) * _PART, :],
                                        ot[:])
    nc.compile()
    return nc


def _run_spmd(nc, in_maps, **kwargs):
    from concourse import bass_utils

    return bass_utils.run_bass_kernel_spmd(
        nc, in_maps, core_ids=list(range(_NCORES)), **kwargs
    )


def kernel(values: np.ndarray, corr: np.ndarray, _collect=None) -> np.ndarray:
    assert values.shape == (_B, _H, _C, _L) and corr.shape == (_B, _H, _C, _L)
    corr16 = np.ascontiguousarray(
        np.asarray(corr, dtype=np.float32).reshape(_B, _R, _L), dtype=np.float16
    )
    vals16 = np.ascontiguousarray(
        np.asarray(values, dtype=np.float32).reshape(_B, _R, _L), dtype=np.float16
    )

    # ---- launch 1: per-batch sums of corr over (H, C) ----
    nc1 = _build_phase1()
    in1 = [
        {"corr_sh": corr16[c * _BLOC:(c + 1) * _BLOC]}
        for c in range(_NCORES)
    ]
    res1 = _run_spmd(nc1, in1, **(_collect.kwargs(1) if _collect else {}))
    if _collect is not None:
        _collect.add(1, nc1, res1)
    sums = np.concatenate(
        [r["sums"].reshape(_BLOC, _L) for r in res1.results], axis=0
    )  # [B, L]

    # ---- host glue: top-k indices + softmax weights (tiny) ----
    mean_value = sums / np.float32(_R)                       # [B, L]
    g = mean_value.astype(np.float64).mean(axis=0)           # [L]
    idx = np.argsort(-g, kind="stable")[:_TOPK].astype(np.int64)
    wsel = mean_value[:, idx].astype(np.float32)             # [B, 6]
    e = np.exp(wsel - wsel.max(axis=-1, keepdims=True))
    w = (e / e.sum(axis=-1, keepdims=True)).astype(np.float32)

    # ---- launch 2: weighted shifted-gather combine ----
    nc2 = _build_phase2([int(i) for i in idx])
    eye = np.eye(_PART, dtype=np.float16)
    in2 = []
    for c in range(_NCORES):
        wloc = w[c * _BLOC:(c + 1) * _BLOC]                  # [BLOC, 6]
        wsb = np.ascontiguousarray(
            np.broadcast_to(wloc.reshape(-1)[None, :], (_PART, _BLOC * _TOPK)),
            dtype=np.float32,
        )
        in2.append({
            "vals": vals16[c * _BLOC:(c + 1) * _BLOC],
            "wsb": wsb,
            "eye": eye,
        })
    res2 = _run_spmd(nc2, in2, **(_collect.kwargs(2) if _collect else {}))
    if _collect is not None:
        _collect.add(2, nc2, res2)
    out = np.concatenate([np.asarray(r["out_sh"]) for r in res2.results], axis=0)
    return out.reshape(_B, _H, _C, _L).astype(np.float32)


# revision 16
# speedup vs baseline: 1.0641x; 1.0506x over previous
"""Trainium2 Bass kernel for Autoformer-style autocorrelation attention.

Math (matches the reference nn.Module):
    top_k = int(log(L)) = 6
    mean_value[b, l] = corr[b].mean(over H, C)                     # [B, L]
    idx = top_k(mean_value.mean(over B))                           # [6]
    w = softmax(mean_value[:, idx], axis=-1)                       # [B, 6]
    out[b, h, c, l] = sum_k w[b, k] * values[b, h, c, (l+idx_k)%L]

Strategy: data-parallel over B (4 batches per core on 8 cores).

Launch 1 reduces corr over (H, C) per batch on-device via ones-matmuls
over the partition axis.  corr is sent as fp16: the quantization error on
the means (~1e-5) is far below the 4.8e-4 top-k selection margin measured
on this distribution, and it halves launch-1 HBM traffic.  The [32, L]
sums return to host, where the tiny top-k + softmax glue runs.

Launch 2 bakes the 6 indices in as static SBUF column windows and emits
the output in fp16 (host casts to fp32; adds <=4.9e-4 relative error
against the 2e-2 gate, and halves the write traffic).  The six shift
terms are split so no engine exceeds the DMA pace: four run on PE as
diag-weighted matmuls accumulating in PSUM, and the last two are fused
into the two DVE scalar_tensor_tensor passes that drain PSUM:

    u16 = (shiftA(v) * wA) + psum      # fp16 out, 2x DVE fast path
    ot  = (shiftB(v) * wB) + u16       # all-SBUF fp16

DVE pieces are split at PSUM bank boundaries (in-bank PSUM reads run
~5x faster than bank-crossing ones) and DVE gets the even shifts.

Diag matrices (w[b,k] * I) are built on-device from a 32KB identity
upload, so launch-2 input DMA is just values fp16 + a few KB.  Per-batch
weights enter through an input tensor so one compiled NEFF is SPMD
across all 8 cores.
"""

import math

import numpy as np

_B, _H, _C, _L = 32, 8, 64, 1024
_NCORES = 8
_BLOC = _B // _NCORES  # batches per core
_R = _H * _C           # rows per batch
_PART = 128
_TPB = _R // _PART     # SBUF tiles per batch
_TOPK = int(math.log(_L))  # 6
_NPE = 5               # shift terms handled by the tensor engine
_HALF = 512            # PSUM bank width in fp32


def _dve_term(idx):
    """The term fused into the DVE drain pass (prefer an even shift)."""
    evens = [k for k in range(_TOPK) if idx[k] % 2 == 0]
    odds = [k for k in range(_TOPK) if idx[k] % 2 == 1]
    return (evens + odds)[0]


def _build_phase1():
    import concourse.bacc as bacc
    import concourse.mybir as mybir
    import concourse.tile as tile

    f32 = mybir.dt.float32
    f16 = mybir.dt.float16
    nc = bacc.Bacc("TRN2", target_bir_lowering=False, debug=False,
                   enable_partition_id=False)
    corr_d = nc.dram_tensor("corr_sh", [_BLOC, _R, _L], f16, kind="ExternalInput").ap()
    sums_d = nc.dram_tensor("sums", [1, _BLOC * _L], f32, kind="ExternalOutput").ap()

    with tile.TileContext(nc) as tc:
        with (
            tc.tile_pool(name="io", bufs=6) as io_pool,
            tc.tile_pool(name="const", bufs=1) as const_pool,
            tc.tile_pool(name="acc", bufs=1) as acc_pool,
            tc.tile_pool(name="ps", bufs=3, space="PSUM") as ps_pool,
        ):
            ones = const_pool.tile([_PART, _HALF], f16)
            nc.vector.memset(ones[:], 1.0)
            outs = acc_pool.tile([1, _BLOC * _L], f32)
            # HAM warmup: a dense junk-matmul burst so the PE clock ramps to
            # full while the entry barrier + first DMA latency play out.  The
            # short [128,128] matmuls keep the activity duty cycle high.
            wps = ps_pool.tile([_PART, _HALF], f32, tag="wps", name="wps", bufs=1)
            for _ in range(6):
                nc.tensor.matmul(wps[:], ones[:, 0:_PART], ones[:],
                                 start=True, stop=True)
            for _ in range(18):
                nc.tensor.matmul(wps[:, 0:_PART], ones[:, 0:_PART],
                                 ones[:, 0:_PART], start=True, stop=True)
            for b in range(_BLOC):
                pss = [ps_pool.tile([_PART, _HALF], f32, tag=f"ps{h}", name=f"ps{h}")
                       for h in range(2)]
                for t2 in range(_TPB // 2):
                    # two row-blocks per DMA: [128, 2L] tile, halving DMA count.
                    # Alternate the trigger engine so the stream rides two
                    # DMA queues (a single queue tops out ~275 GB/s); gpsimd
                    # (SWDGE) is otherwise idle in this phase.
                    eng = nc.sync if (b * (_TPB // 2) + t2) % 2 == 0 else nc.gpsimd
                    vt = io_pool.tile([_PART, 2 * _L], f16, tag="vt")
                    src_ap = corr_d[b, t2 * 2 * _PART:(t2 + 1) * 2 * _PART, :]
                    eng.dma_start(
                        vt[:].rearrange("p (u l) -> p u l", u=2),
                        src_ap.rearrange("(u p) l -> p u l", p=_PART))
                    for u in range(2):
                        for h in range(2):
                            nc.tensor.matmul(
                                pss[h][:],
                                ones[:, 0:_PART],
                                vt[:, u * _L + h * _HALF:u * _L + (h + 1) * _HALF],
                                start=(t2 == 0 and u == 0),
                                stop=(t2 == _TPB // 2 - 1 and u == 1),
                            )
                for h in range(2):
                    o0 = b * _L + h * _HALF
                    nc.scalar.copy(outs[0:1, o0:o0 + _HALF], pss[h][0:1, :])
                nc.scalar.dma_start(
                    sums_d[0:1, b * _L:(b + 1) * _L],
                    outs[0:1, b * _L:(b + 1) * _L])
    nc.compile()
    return nc


def _wrap_pieces(s):
    """Split the circular window [s, s+L) into contiguous source pieces.

    Returns [(dst_off, n, src_off), ...] with sum(n) == L.
    """
    if s == 0:
        return [(0, _L, 0)]
    return [(0, _L - s, s), (_L - s, s, 0)]


def _build_phase2(idx):
    import concourse.bacc as bacc
    import concourse.mybir as mybir
    import concourse.tile as tile

    f32 = mybir.dt.float32
    f16 = mybir.dt.float16
    alu = mybir.AluOpType

    # Five terms run on PE as diag-matmuls; the remaining one is fused into
    # the single DVE drain pass (DVE instructions cost ~330ns fixed each, so
    # the drain must be as few pieces as possible).  DVE's fp16 2x fast path
    # prefers even source offsets, so give DVE an even shift if available.
    kd = _dve_term(idx)
    kpe = [k for k in range(_TOPK) if k != kd]
    assert len(kpe) == _NPE

    nc = bacc.Bacc("TRN2", target_bir_lowering=False, debug=False,
                   enable_partition_id=False)
    vals_d = nc.dram_tensor("vals", [_BLOC, _R, _L], f16, kind="ExternalInput").ap()
    wsb_d = nc.dram_tensor("wsb", [_PART, _BLOC * _TOPK], f32, kind="ExternalInput").ap()
    diag_d = nc.dram_tensor(
        "diags", [_PART, _BLOC * _NPE * _PART], f16, kind="ExternalInput").ap()
    out_d = nc.dram_tensor("out_sh", [_BLOC, _R, _L], f16, kind="ExternalOutput").ap()

    with tile.TileContext(nc) as tc:
        with (
            tc.tile_pool(name="const", bufs=1) as const_pool,
            tc.tile_pool(name="v16", bufs=10) as v16_pool,
            tc.tile_pool(name="out", bufs=4) as out_pool,
            tc.tile_pool(name="ps", bufs=4, space="PSUM") as ps_pool,
        ):
            # consts ride the otherwise-idle gpsimd SWDGE queue so they never
            # delay the values stream on the sync queue
            w_t = const_pool.tile([_PART, _BLOC * _TOPK], f32)
            nc.gpsimd.dma_start(w_t[:], wsb_d[:])
            diag = const_pool.tile([_PART, _BLOC * _NPE * _PART], f16)
            nc.gpsimd.dma_start(diag[:], diag_d[:])
            # HAM warmup: a dense burst of junk matmuls (a couple of full-bank
            # ones plus many short ones) keeps the PE duty cycle high so the
            # clock ramps to full before the real stream begins.
            wones = const_pool.tile([_PART, _HALF], f16)
            nc.vector.memset(wones[:], 1.0)
            wp = ps_pool.tile([_PART, _HALF], f32, tag="psA", name="wm")
            for _ in range(2):
                nc.tensor.matmul(wp[:], wones[:, 0:_PART], wones[:],
                                 start=True, stop=True)
            for _ in range(20):
                nc.tensor.matmul(wp[:, 0:_PART], wones[:, 0:_PART],
                                 wones[:, 0:_PART], start=True, stop=True)
            for b in range(_BLOC):
                wd = w_t[:, b * _TOPK + kd:b * _TOPK + kd + 1]
                for t in range(_TPB):
                    vt16 = v16_pool.tile([_PART, _L], f16, tag="vt16")
                    nc.sync.dma_start(
                        vt16[:], vals_d[b, t * _PART:(t + 1) * _PART, :])

                    # PE: 5 diag-weighted shift terms accumulate per PSUM
                    # bank; bank A pieces first so DVE can drain bank A while
                    # PE still works on bank B.  Per-bank PSUM tiles: APs at
                    # offsets >=2KB into a PSUM tile read ~3x slower on DVE.
                    pss = [ps_pool.tile([_PART, _HALF], f32, tag=f"ps{hn}",
                                        name=f"ps{hn}")
                           for hn in ("A", "B")]
                    for h in range(2):
                        pieces = []
                        for j, k in enumerate(kpe):
                            dof = (b * _NPE + j) * _PART
                            s = (idx[k] + h * _HALF) % _L
                            n1 = min(_HALF, _L - s)
                            pieces.append((dof, 0, n1, s))
                            if n1 < _HALF:
                                pieces.append((dof, n1, _HALF - n1, 0))
                        for pi, (dof, o0, n, s) in enumerate(pieces):
                            nc.tensor.matmul(
                                pss[h][:, o0:o0 + n], diag[:, dof:dof + _PART],
                                vt16[:, s:s + n],
                                start=(pi == 0), stop=(pi == len(pieces) - 1),
                            )

                    # DVE: single fused drain pass per tile:
                    #   ot = (shift_kd(v) * wd) + psum   (fp16 out)
                    ot = out_pool.tile([_PART, _L], f16, tag="ot")
                    sd = idx[kd]
                    for h in range(2):
                        q = (sd + h * _HALF) % _L
                        n1 = min(_HALF, _L - q)
                        segs = [(h * _HALF, n1, q)]
                        if n1 < _HALF:
                            segs.append((h * _HALF + n1, _HALF - n1, 0))
                        for (d0, n, s0) in segs:
                            nc.vector.scalar_tensor_tensor(
                                ot[:, d0:d0 + n], vt16[:, s0:s0 + n], wd,
                                pss[h][:, d0 - h * _HALF:d0 - h * _HALF + n],
                                op0=alu.mult, op1=alu.add)
                    nc.scalar.dma_start(out_d[b, t * _PART:(t + 1) * _PART, :],
                                        ot[:])
    nc.compile()
    return nc


def _run_spmd(nc, in_maps, **kwargs):
    from concourse import bass_utils

    return bass_utils.run_bass_kernel_spmd(
        nc, in_maps, core_ids=list(range(_NCORES)), **kwargs
    )


def kernel(values: np.ndarray, corr: np.ndarray, _collect=None) -> np.ndarray:
    assert values.shape == (_B, _H, _C, _L) and corr.shape == (_B, _H, _C, _L)
    corr16 = np.ascontiguousarray(
        np.asarray(corr, dtype=np.float32).reshape(_B, _R, _L), dtype=np.float16
    )
    vals16 = np.ascontiguousarray(
        np.asarray(values, dtype=np.float32).reshape(_B, _R, _L), dtype=np.float16
    )

    # ---- launch 1: per-batch sums of corr over (H, C) ----
    nc1 = _build_phase1()
    in1 = [
        {"corr_sh": corr16[c * _BLOC:(c + 1) * _BLOC]}
        for c in range(_NCORES)
    ]
    res1 = _run_spmd(nc1, in1, **(_collect.kwargs(1) if _collect else {}))
    if _collect is not None:
        _collect.add(1, nc1, res1)
    sums = np.concatenate(
        [r["sums"].reshape(_BLOC, _L) for r in res1.results], axis=0
    )  # [B, L]

    # ---- host glue: top-k indices + softmax weights (tiny) ----
    mean_value = sums / np.float32(_R)                       # [B, L]
    g = mean_value.astype(np.float64).mean(axis=0)           # [L]
    idx = np.argsort(-g, kind="stable")[:_TOPK].astype(np.int64)
    wsel = mean_value[:, idx].astype(np.float32)             # [B, 6]
    e = np.exp(wsel - wsel.max(axis=-1, keepdims=True))
    w = (e / e.sum(axis=-1, keepdims=True)).astype(np.float32)

    # ---- launch 2: weighted shifted-gather combine ----
    idx_l = [int(i) for i in idx]
    nc2 = _build_phase2(idx_l)
    kd = _dve_term(idx_l)
    kpe = [k for k in range(_TOPK) if k != kd]
    eye = np.eye(_PART, dtype=np.float16)
    in2 = []
    for c in range(_NCORES):
        wloc = w[c * _BLOC:(c + 1) * _BLOC]                  # [BLOC, 6]
        wsb = np.ascontiguousarray(
            np.broadcast_to(wloc.reshape(-1)[None, :], (_PART, _BLOC * _TOPK)),
            dtype=np.float32,
        )
        diags = np.concatenate(
            [eye * np.float16(wloc[b, k]) for b in range(_BLOC) for k in kpe],
            axis=1,
        )  # [128, BLOC*NPE*128] fp16
        in2.append({
            "vals": vals16[c * _BLOC:(c + 1) * _BLOC],
            "wsb": wsb,
            "diags": np.ascontiguousarray(diags),
        })
    res2 = _run_spmd(nc2, in2, **(_collect.kwargs(2) if _collect else {}))
    if _collect is not None:
        _collect.add(2, nc2, res2)
    out = np.concatenate([np.asarray(r["out_sh"]) for r in res2.results], axis=0)
    return out.reshape(_B, _H, _C, _L).astype(np.float32)


# revision 19
# speedup vs baseline: 1.0907x; 1.0249x over previous
"""Trainium2 Bass kernel for Autoformer-style autocorrelation attention.

Math (matches the reference nn.Module):
    top_k = int(log(L)) = 6
    mean_value[b, l] = corr[b].mean(over H, C)                     # [B, L]
    idx = top_k(mean_value.mean(over B))                           # [6]
    w = softmax(mean_value[:, idx], axis=-1)                       # [B, 6]
    out[b, h, c, l] = sum_k w[b, k] * values[b, h, c, (l+idx_k)%L]

Strategy: data-parallel over B (4 batches per core on 8 cores).

Launch 1 reduces corr over (H, C) per batch on-device via ones-matmuls
over the partition axis.  corr is sent as fp16: the quantization error on
the means (~1e-5) is far below the 4.8e-4 top-k selection margin measured
on this distribution, and it halves launch-1 HBM traffic.  The [32, L]
sums return to host, where the tiny top-k + softmax glue runs.

Launch 2 bakes the 6 indices in as static SBUF column windows and emits
the output in fp16 (host casts to fp32; adds <=4.9e-4 relative error
against the 2e-2 gate, and halves the write traffic).  The six shift
terms are split so no engine exceeds the DMA pace: four run on PE as
diag-weighted matmuls accumulating in PSUM, and the last two are fused
into the two DVE scalar_tensor_tensor passes that drain PSUM:

    u16 = (shiftA(v) * wA) + psum      # fp16 out, 2x DVE fast path
    ot  = (shiftB(v) * wB) + u16       # all-SBUF fp16

DVE pieces are split at PSUM bank boundaries (in-bank PSUM reads run
~5x faster than bank-crossing ones) and DVE gets the even shifts.

Diag matrices (w[b,k] * I) are built on-device from a 32KB identity
upload, so launch-2 input DMA is just values fp16 + a few KB.  Per-batch
weights enter through an input tensor so one compiled NEFF is SPMD
across all 8 cores.
"""

import math

import numpy as np

_B, _H, _C, _L = 32, 8, 64, 1024
_NCORES = 8
_BLOC = _B // _NCORES  # batches per core
_R = _H * _C           # rows per batch
_PART = 128
_TPB = _R // _PART     # SBUF tiles per batch
_TOPK = int(math.log(_L))  # 6
_NPE = 5               # shift terms handled by the tensor engine
_HALF = 512            # PSUM bank width in fp32


def _dve_term(idx):
    """The term fused into the DVE drain pass (prefer an even shift)."""
    evens = [k for k in range(_TOPK) if idx[k] % 2 == 0]
    odds = [k for k in range(_TOPK) if idx[k] % 2 == 1]
    return (evens + odds)[0]


def _build_phase1():
    import concourse.bacc as bacc
    import concourse.mybir as mybir
    import concourse.tile as tile

    f32 = mybir.dt.float32
    f16 = mybir.dt.float16
    nc = bacc.Bacc("TRN2", target_bir_lowering=False, debug=False,
                   enable_partition_id=False)
    corr_d = nc.dram_tensor("corr_sh", [_BLOC, _R, _L], f16, kind="ExternalInput").ap()
    sums_d = nc.dram_tensor("sums", [1, _BLOC * _L], f32, kind="ExternalOutput").ap()

    with tile.TileContext(nc) as tc:
        with (
            tc.tile_pool(name="io", bufs=6) as io_pool,
            tc.tile_pool(name="const", bufs=1) as const_pool,
            tc.tile_pool(name="acc", bufs=1) as acc_pool,
            tc.tile_pool(name="ps", bufs=3, space="PSUM") as ps_pool,
        ):
            ones = const_pool.tile([_PART, _HALF], f16)
            nc.vector.memset(ones[:], 1.0)
            outs = acc_pool.tile([1, _BLOC * _L], f32)
            # HAM warmup: a dense junk-matmul burst so the PE clock ramps to
            # full while the entry barrier + first DMA latency play out.  The
            # short [128,128] matmuls keep the activity duty cycle high.
            wps = ps_pool.tile([_PART, _HALF], f32, tag="wps", name="wps", bufs=1)
            for _ in range(4):
                nc.tensor.matmul(wps[:], ones[:, 0:_PART], ones[:],
                                 start=True, stop=True)
            for _ in range(8):
                nc.tensor.matmul(wps[:, 0:_PART], ones[:, 0:_PART],
                                 ones[:, 0:_PART], start=True, stop=True)
            for b in range(_BLOC):
                pss = [ps_pool.tile([_PART, _HALF], f32, tag=f"ps{h}", name=f"ps{h}")
                       for h in range(2)]
                for t2 in range(_TPB // 2):
                    # two row-blocks per DMA: [128, 2L] tile, halving DMA count.
                    # Alternate the trigger engine so the stream rides two
                    # DMA queues (a single queue tops out ~275 GB/s); gpsimd
                    # (SWDGE) is otherwise idle in this phase.
                    eng = nc.sync if (b * (_TPB // 2) + t2) % 2 == 0 else nc.gpsimd
                    vt = io_pool.tile([_PART, 2 * _L], f16, tag="vt")
                    src_ap = corr_d[b, t2 * 2 * _PART:(t2 + 1) * 2 * _PART, :]
                    eng.dma_start(
                        vt[:].rearrange("p (u l) -> p u l", u=2),
                        src_ap.rearrange("(u p) l -> p u l", p=_PART))
                    for u in range(2):
                        for h in range(2):
                            nc.tensor.matmul(
                                pss[h][:],
                                ones[:, 0:_PART],
                                vt[:, u * _L + h * _HALF:u * _L + (h + 1) * _HALF],
                                start=(t2 == 0 and u == 0),
                                stop=(t2 == _TPB // 2 - 1 and u == 1),
                            )
                # drain the two PSUM banks in parallel on ACT and DVE
                o0 = b * _L
                nc.scalar.copy(outs[0:1, o0:o0 + _HALF], pss[0][0:1, :])
                nc.vector.tensor_scalar_mul(
                    outs[0:1, o0 + _HALF:o0 + _L], pss[1][0:1, :], 1.0)
                nc.scalar.dma_start(
                    sums_d[0:1, b * _L:(b + 1) * _L],
                    outs[0:1, b * _L:(b + 1) * _L])
    nc.compile()
    return nc


def _wrap_pieces(s):
    """Split the circular window [s, s+L) into contiguous source pieces.

    Returns [(dst_off, n, src_off), ...] with sum(n) == L.
    """
    if s == 0:
        return [(0, _L, 0)]
    return [(0, _L - s, s), (_L - s, s, 0)]


def _build_phase2(idx):
    import concourse.bacc as bacc
    import concourse.mybir as mybir
    import concourse.tile as tile

    f32 = mybir.dt.float32
    f16 = mybir.dt.float16
    alu = mybir.AluOpType

    # Five terms run on PE as diag-matmuls; the remaining one is fused into
    # the single DVE drain pass (DVE instructions cost ~330ns fixed each, so
    # the drain must be as few pieces as possible).  DVE's fp16 2x fast path
    # prefers even source offsets, so give DVE an even shift if available.
    kd = _dve_term(idx)
    kpe = [k for k in range(_TOPK) if k != kd]
    assert len(kpe) == _NPE

    nc = bacc.Bacc("TRN2", target_bir_lowering=False, debug=False,
                   enable_partition_id=False)
    vals_d = nc.dram_tensor("vals", [_BLOC, _R, _L], f16, kind="ExternalInput").ap()
    wsb_d = nc.dram_tensor("wsb", [_PART, _BLOC * _TOPK], f32, kind="ExternalInput").ap()
    diag_d = nc.dram_tensor(
        "diags", [_PART, _BLOC * _NPE * _PART], f16, kind="ExternalInput").ap()
    out_d = nc.dram_tensor("out_sh", [_BLOC, _R, _L], f16, kind="ExternalOutput").ap()

    with tile.TileContext(nc) as tc:
        with (
            tc.tile_pool(name="const", bufs=1) as const_pool,
            tc.tile_pool(name="v16", bufs=10) as v16_pool,
            tc.tile_pool(name="out", bufs=4) as out_pool,
            tc.tile_pool(name="ps", bufs=4, space="PSUM") as ps_pool,
        ):
            # consts ride the scalar HWDGE queue (idle until outputs start
            # ~14us in) so they never delay the values stream on the sync
            # queue; diags are split per batch so batch 0's stationaries land
            # early -- a late diag stalls PE and defers the HAM clock ramp.
            w_t = const_pool.tile([_PART, _BLOC * _TOPK], f32)
            nc.scalar.dma_start(w_t[:], wsb_d[:])
            diag = const_pool.tile([_PART, _BLOC * _NPE * _PART], f16)
            dstride = _NPE * _PART
            for b in range(_BLOC):
                nc.scalar.dma_start(
                    diag[:, b * dstride:(b + 1) * dstride],
                    diag_d[:, b * dstride:(b + 1) * dstride])
            # HAM warmup: junk matmuls ramp the PE clock while the entry
            # barrier + first DMA latency play out; kept short because they
            # share the PE queue with (and thus delay) the real stream.
            wones = const_pool.tile([_PART, _HALF], f16)
            nc.vector.memset(wones[:], 1.0)
            wp = ps_pool.tile([_PART, _HALF], f32, tag="psA", name="wm")
            for _ in range(2):
                nc.tensor.matmul(wp[:], wones[:, 0:_PART], wones[:],
                                 start=True, stop=True)
            for _ in range(10):
                nc.tensor.matmul(wp[:, 0:_PART], wones[:, 0:_PART],
                                 wones[:, 0:_PART], start=True, stop=True)
            for b in range(_BLOC):
                wd = w_t[:, b * _TOPK + kd:b * _TOPK + kd + 1]
                for t in range(_TPB):
                    vt16 = v16_pool.tile([_PART, _L], f16, tag="vt16")
                    nc.sync.dma_start(
                        vt16[:], vals_d[b, t * _PART:(t + 1) * _PART, :])

                    # PE: 5 diag-weighted shift terms accumulate per PSUM
                    # bank; bank A pieces first so DVE can drain bank A while
                    # PE still works on bank B.  Per-bank PSUM tiles: APs at
                    # offsets >=2KB into a PSUM tile read ~3x slower on DVE.
                    pss = [ps_pool.tile([_PART, _HALF], f32, tag=f"ps{hn}",
                                        name=f"ps{hn}")
                           for hn in ("A", "B")]
                    for h in range(2):
                        pieces = []
                        for j, k in enumerate(kpe):
                            dof = (b * _NPE + j) * _PART
                            s = (idx[k] + h * _HALF) % _L
                            n1 = min(_HALF, _L - s)
                            pieces.append((dof, 0, n1, s))
                            if n1 < _HALF:
                                pieces.append((dof, n1, _HALF - n1, 0))
                        for pi, (dof, o0, n, s) in enumerate(pieces):
                            nc.tensor.matmul(
                                pss[h][:, o0:o0 + n], diag[:, dof:dof + _PART],
                                vt16[:, s:s + n],
                                start=(pi == 0), stop=(pi == len(pieces) - 1),
                            )

                    # DVE: single fused drain pass per tile:
                    #   ot = (shift_kd(v) * wd) + psum   (fp16 out)
                    ot = out_pool.tile([_PART, _L], f16, tag="ot")
                    sd = idx[kd]
                    for h in range(2):
                        q = (sd + h * _HALF) % _L
                        n1 = min(_HALF, _L - q)
                        segs = [(h * _HALF, n1, q)]
                        if n1 < _HALF:
                            segs.append((h * _HALF + n1, _HALF - n1, 0))
                        for (d0, n, s0) in segs:
                            nc.vector.scalar_tensor_tensor(
                                ot[:, d0:d0 + n], vt16[:, s0:s0 + n], wd,
                                pss[h][:, d0 - h * _HALF:d0 - h * _HALF + n],
                                op0=alu.mult, op1=alu.add)
                    nc.scalar.dma_start(out_d[b, t * _PART:(t + 1) * _PART, :],
                                        ot[:])
    nc.compile()
    return nc


def _run_spmd(nc, in_maps, **kwargs):
    from concourse import bass_utils

    return bass_utils.run_bass_kernel_spmd(
        nc, in_maps, core_ids=list(range(_NCORES)), **kwargs
    )


def kernel(values: np.ndarray, corr: np.ndarray, _collect=None) -> np.ndarray:
    assert values.shape == (_B, _H, _C, _L) and corr.shape == (_B, _H, _C, _L)
    corr16 = np.ascontiguousarray(
        np.asarray(corr, dtype=np.float32).reshape(_B, _R, _L), dtype=np.float16
    )
    vals16 = np.ascontiguousarray(
        np.asarray(values, dtype=np.float32).reshape(_B, _R, _L), dtype=np.float16
    )

    # ---- launch 1: per-batch sums of corr over (H, C) ----
    nc1 = _build_phase1()
    in1 = [
        {"corr_sh": corr16[c * _BLOC:(c + 1) * _BLOC]}
        for c in range(_NCORES)
    ]
    res1 = _run_spmd(nc1, in1, **(_collect.kwargs(1) if _collect else {}))
    if _collect is not None:
        _collect.add(1, nc1, res1)
    sums = np.concatenate(
        [r["sums"].reshape(_BLOC, _L) for r in res1.results], axis=0
    )  # [B, L]

    # ---- host glue: top-k indices + softmax weights (tiny) ----
    mean_value = sums / np.float32(_R)                       # [B, L]
    g = mean_value.astype(np.float64).mean(axis=0)           # [L]
    idx = np.argsort(-g, kind="stable")[:_TOPK].astype(np.int64)
    wsel = mean_value[:, idx].astype(np.float32)             # [B, 6]
    e = np.exp(wsel - wsel.max(axis=-1, keepdims=True))
    w = (e / e.sum(axis=-1, keepdims=True)).astype(np.float32)

    # ---- launch 2: weighted shifted-gather combine ----
    idx_l = [int(i) for i in idx]
    nc2 = _build_phase2(idx_l)
    kd = _dve_term(idx_l)
    kpe = [k for k in range(_TOPK) if k != kd]
    eye = np.eye(_PART, dtype=np.float16)
    in2 = []
    for c in range(_NCORES):
        wloc = w[c * _BLOC:(c + 1) * _BLOC]                  # [BLOC, 6]
        wsb = np.ascontiguousarray(
            np.broadcast_to(wloc.reshape(-1)[None, :], (_PART, _BLOC * _TOPK)),
            dtype=np.float32,
        )
        diags = np.concatenate(
            [eye * np.float16(wloc[b, k]) for b in range(_BLOC) for k in kpe],
            axis=1,
        )  # [128, BLOC*NPE*128] fp16
        in2.append({
            "vals": vals16[c * _BLOC:(c + 1) * _BLOC],
            "wsb": wsb,
            "diags": np.ascontiguousarray(diags),
        })
    res2 = _run_spmd(nc2, in2, **(_collect.kwargs(2) if _collect else {}))
    if _collect is not None:
        _collect.add(2, nc2, res2)
    out = np.concatenate([np.asarray(r["out_sh"]) for r in res2.results], axis=0)
    return out.reshape(_B, _H, _C, _L).astype(np.float32)
